# revision 31
# baseline (speedup 1.0000x reference)
"""Head-sharded (tensor-parallel) causal attention block for 8 NeuronCores.

Model: B=2, S=2048, D=1024, H=16 heads (HD=64). Each core owns 2 heads
(128 features) of the QKV projections and attention, computes a partial
output projection (o_shard @ ow_shard) in fp32, and the host sums the 8
partials and adds the output bias.

Bias algebra (host-side folding):
  - k bias: softmax over keys is invariant to per-query constants;
    (q+qb)·(k+kb) = (q+qb)·k + const(q) -> kb dropped entirely.
  - v bias: softmax rows sum to 1, so o = sum p (v+vb) = sum p v + vb;
    after the output projection that is a constant vector ow @ vb folded
    into the output bias on the host. -> vb dropped entirely.
  Only qb is applied on-device (fused into the q PSUM->SBUF copy).

Per-core kernel phases (software-pipelined; attention chunk i only needs
projection chunks <= i, so proj/outproj matmuls are interleaved into the
attention j-loops via a background-work queue):
  1. Projections from xT chunks: qT/kT [feat 128, seq] accumulate 8
     D-blocks in PSUM (q: bias fused in the DVE copy; k: plain copy on
     Pool). V is computed directly token-major: V[t 128, feat 128] per
     t-block (lhsT = xT block, rhs = vwT block), written into
     v_aug[t, b, j, 0:64 | 65:129] with column 64 = 1.0 (the ones column
     makes the PV matmul also produce the softmax denominator).
  2. Attention per (batch, 512-query chunk): scores transposed
     sT[t,sq] = K@Q.T, two heads in the two PSUM banks of one tile; one
     exp on ScalarE covers both banks (scale=1/8 folded in). Causal mask
     applied multiplicatively (0/1 fp16) AFTER exp on the Pool engine,
     only on the boundary 128-col strip of diagonal t-blocks. PV is
     token-major: po_pack[s 128, si, h, 65] accumulates pt_block.T @
     v_aug per s-block with exact causal bounds (j <= 4i+si); the packed
     accumulators share banks, so the tile is pre-zeroed and matmuls use
     start=False (PSUM start=True zeroes whole 2KB bank regions).
  3. Normalize: per-partition reciprocal + tensor_scalar_mul (DVE), then
     a PE transpose (fp32 identity) restores feature-major oT_sb.
  4. Output projection: partial[sq,1024] in two 512-wide fp32 PSUM
     tiles; PSUM->SBUF cast copies alternate between DVE and Pool, then
     one fp16 DMA per half-row-block.

Matmul inputs are fp16 (full-rate on the PE).
"""

import numpy as np

import concourse.bass as bass
import concourse.mybir as mybir
import concourse.tile as tile
from concourse import bacc
from concourse.bass import ts
from concourse.bass_utils import run_bass_kernel_spmd
from concourse.masks import make_identity

B, S, D, H = 2, 2048, 1024, 16
HD = D // H            # 64 head dim
NCORES = 8
FPC = D // NCORES      # 128 features per core
HPC = FPC // HD        # 2 heads per core
P = 128
SQ_CHUNK = 512         # query chunk (matmul free dim)
NSQ = S // SQ_CHUNK    # 4
NTB = S // P           # 16 t-blocks
DBLK = D // P          # 8 contraction blocks for projections

F32 = mybir.dt.float32
MM_DT = mybir.dt.float16
FP8 = mybir.dt.float8e4
_NP_MM = "float16"
FP8_QK = False         # q/k projections in fp8e4 DoubleRow (weights x64)
W8_SCALE = 64.0
EXP_SCALE = 0.125 / (W8_SCALE * W8_SCALE) if FP8_QK else 0.125

_module_cache = {}


def _build_module(repeat=1):
    nc = bacc.Bacc("TRN2", target_bir_lowering=False, debug=False)

    xT_d = nc.dram_tensor("xT", [B, D, S], MM_DT, kind="ExternalInput").ap()
    wT_d = nc.dram_tensor("wT", [D, 3 * FPC], MM_DT, kind="ExternalInput").ap()
    if FP8_QK:
        x8_d = nc.dram_tensor("x8", [B, D, S], FP8, kind="ExternalInput").ap()
        w8_d = nc.dram_tensor("w8", [D, 2 * FPC], FP8, kind="ExternalInput").ap()
    qb_d = nc.dram_tensor("qb", [FPC, 1], F32, kind="ExternalInput").ap()
    owT_d = nc.dram_tensor("owT", [FPC, D], MM_DT, kind="ExternalInput").ap()
    maskm_d = nc.dram_tensor("maskm", [P, 3 * P], MM_DT, kind="ExternalInput").ap()
    out_d = nc.dram_tensor("out", [B, S, D], MM_DT, kind="ExternalOutput").ap()

    # [B, D, S] with D split into 8 blocks of 128 partitions
    xT_r = xT_d.rearrange("b (o p) s -> b p o s", p=P)
    x8_r = x8_d.rearrange("b (o p) s -> b p o s", p=P) if FP8_QK else None

    with tile.TileContext(nc) as tc:
        with (
            tc.tile_pool(name="singles", bufs=1) as singles,
            tc.tile_pool(name="xin", bufs=4) as xin,
            tc.tile_pool(name="ptile", bufs=6) as ptile,
            tc.tile_pool(name="small", bufs=6) as small,
        ):
            # --- constants / persistent tensors ---
            wT_sb = singles.tile([P, DBLK, 3 * FPC], MM_DT, tag="w")
            wT_r = wT_d.rearrange("(o p) m -> p o m", p=P)
            nc.sync.dma_start(out=wT_sb[:, :, 0:FPC], in_=wT_r[:, :, 0:FPC])
            qwT_sb = wT_sb[:, :, 0:FPC]
            kwT_sb = wT_sb[:, :, FPC:2 * FPC]
            vwT_sb = wT_sb[:, :, 2 * FPC:3 * FPC]
            if FP8_QK:
                w8_sb = singles.tile([P, DBLK, 2 * FPC], FP8, tag="w8")
                nc.sync.dma_start(
                    out=w8_sb, in_=w8_d.rearrange("(o p) m -> p o m", p=P))
            else:
                w8_sb = None
            qb_sb = singles.tile([FPC, 1], F32, tag="qb")
            owT_sb = singles.tile([FPC, D], MM_DT, tag="ow")
            maskm_sb = singles.tile([P, 3 * P], MM_DT, tag="maskm")

            def emit_consts():
                nc.scalar.dma_start(out=qb_sb, in_=qb_d)
                nc.scalar.dma_start(out=wT_sb[:, :, FPC:], in_=wT_r[:, :, FPC:])
                nc.scalar.dma_start(out=owT_sb, in_=owT_d)
                nc.scalar.dma_start(out=maskm_sb, in_=maskm_d)
            ident = singles.tile([P, P], F32, tag="ident")
            make_identity(nc, ident)

            qT_sb = singles.tile([P, B, S], MM_DT, tag="qT")
            kT_sb = singles.tile([P, B, S], MM_DT, tag="kT")
            oT_sb = singles.tile([P, B, S], MM_DT, tag="oT")
            # V_aug[t, b, h, tblk, 0:64] = v feats; [.., 64] = 1.0 (the
            # ones column makes the PV matmul also emit the denominator
            # in po row 64 for both heads).
            v_aug = singles.tile([P, B, HPC, NTB, HD + 1], MM_DT, tag="vaug")
            ones_sb = singles.tile([P, 1], F32, tag="ones")
            nc.vector.memset(ones_sb, 1.0)
            nc.vector.tensor_copy(
                out=v_aug[:, :, :, :, HD],
                in_=ones_sb[:, 0][:, None, None, None].to_broadcast([P, B, HPC, NTB]),
            )

            # ---------- repetitions (>1 only for HW timing calibration) ---
            for _rep in range(repeat):
                _emit_body(nc, tc, locals(), emit_consts if _rep == 0 else None)

    return nc


def _emit_body(nc, tc, env, emit_consts=None):
    g = type("G", (), env)
    xin, ptile, small = g.xin, g.ptile, g.small
    qwT_sb, kwT_sb, vwT_sb, w8_sb = g.qwT_sb, g.kwT_sb, g.vwT_sb, g.w8_sb
    x8_r = g.x8_r
    qb_sb, owT_sb, maskm_sb = g.qb_sb, g.owT_sb, g.maskm_sb
    qT_sb, kT_sb, oT_sb, v_aug = g.qT_sb, g.kT_sb, g.oT_sb, g.v_aug
    ident = g.ident
    xT_r, out_d = g.xT_r, g.out_d

    # PSUM budget (8 banks): pj 2x[P,512] (proj accum, then outproj) = 2,
    # mp 2x[P,2,512] (QK scores) = 4, po 2x[65,512] (PV accum) = 2.
    with (
        tc.tile_pool(name="outsb", bufs=4) as outsb,
        tc.tile_pool(name="pj", bufs=2, space="PSUM") as pj,
        tc.tile_pool(name="mp", bufs=2, space="PSUM") as mp,
        tc.tile_pool(name="po", bufs=1, space="PSUM") as po,
    ):
        # ---- background work queue: each item emits a small batch of
        # independent PE work (proj piece / outproj piece); popped between
        # attention j-steps to fill PE stalls while ScalarE does exp.
        bg = []

        def drain_bg(n):
            for _ in range(n):
                if bg:
                    bg.pop(0)()

        def emit_xt_load(b, cn, split=False):
            x8t = None
            if FP8_QK:
                x8t = xin.tile([P, DBLK, SQ_CHUNK], FP8, tag="x8t",
                               name=f"x8t{b}{cn}")
                nc.sync.dma_start(
                    out=x8t, in_=x8_r[b, :, :, ts(cn, SQ_CHUNK)],
                )
            xt = xin.tile([P, DBLK, SQ_CHUNK], MM_DT, tag="xt",
                          name=f"xt{b}{cn}")
            if split:
                nc.scalar.dma_start(
                    out=xt[:, 0:2, :],
                    in_=xT_r[b, :, 0:2, ts(cn, SQ_CHUNK)],
                )
                for q in range(2, DBLK, 2):
                    nc.sync.dma_start(
                        out=xt[:, q:q + 2, :],
                        in_=xT_r[b, :, q:q + 2, ts(cn, SQ_CHUNK)],
                    )
            else:
                nc.sync.dma_start(
                    out=xt, in_=xT_r[b, :, :, ts(cn, SQ_CHUNK)],
                )
            return xt, x8t

        def emit_qk_piece(b, cn, xtpair, kind):
            xt, x8t = xtpair
            ps = pj.tile([P, SQ_CHUNK], F32, tag="pj", name=f"prj{b}{cn}{kind}")
            if FP8_QK:
                f0 = 0 if kind == "q" else FPC
                for op in range(DBLK // 2):
                    nc.tensor.matmul(
                        ps,
                        lhsT=w8_sb[:, 2 * op:2 * op + 2, f0:f0 + FPC],
                        rhs=x8t[:, 2 * op:2 * op + 2, :],
                        start=(op == 0),
                        stop=(op == DBLK // 2 - 1),
                        perf_mode=mybir.MatmulPerfMode.DoubleRow,
                    )
            else:
                wT_sb = qwT_sb if kind == "q" else kwT_sb
                for o in range(DBLK):
                    nc.tensor.matmul(
                        ps,
                        lhsT=wT_sb[:, o, :],
                        rhs=xt[:, o, :],
                        start=(o == 0),
                        stop=(o == DBLK - 1),
                    )
            if kind == "q":
                nc.vector.tensor_scalar_add(
                    out=qT_sb[:, b, ts(cn, SQ_CHUNK)], in0=ps, scalar1=qb_sb,
                )
            else:
                nc.scalar.copy(
                    out=kT_sb[:, b, ts(cn, SQ_CHUNK)], in_=ps,
                )

        def emit_v_piece(b, cn, xtpair, jj):
            xt = xtpair[0]
            # token-major V for t-block j: [t 128, feat 128]
            j = 4 * cn + jj
            vps = pj.tile([P, FPC], F32, tag="pj", name=f"vp{b}{j}")
            toff = jj * P
            for o in range(DBLK):
                nc.tensor.matmul(
                    vps,
                    lhsT=xt[:, o, toff:toff + P],
                    rhs=vwT_sb[:, o, :],
                    start=(o == 0),
                    stop=(o == DBLK - 1),
                )
            nc.vector.tensor_copy(out=v_aug[:, b, 0, j, 0:HD], in_=vps[:, 0:HD])
            nc.vector.tensor_copy(
                out=v_aug[:, b, 1, j, 0:HD], in_=vps[:, HD:FPC],
            )

        def queue_proj_chunk(b, cn, split=False):
            xt_box = []

            def load():
                xt_box.append(emit_xt_load(b, cn, split))

            bg.append(load)
            bg.append(lambda: emit_qk_piece(b, cn, xt_box[0], "q"))
            bg.append(lambda: emit_qk_piece(b, cn, xt_box[0], "k"))
            for jj in range(4):
                bg.append(lambda jj=jj: emit_v_piece(b, cn, xt_box[0], jj))

        def emit_outproj_tail(b, s):
            pp = mp.tile([P, HPC, SQ_CHUNK], F32, tag="mp", name=f"ppt{b}_{s}")
            for cc in range(2):
                nc.tensor.matmul(
                    pp[:, cc, :],
                    lhsT=oT_sb[:, b, ts(s, P)],
                    rhs=owT_sb[:, ts(cc, SQ_CHUNK)],
                    start=True,
                    stop=True,
                )
            ot = outsb.tile([P, D], MM_DT, tag="ot", name=f"ott{b}_{s}")
            nc.vector.tensor_copy(out=ot[:, 0:SQ_CHUNK], in_=pp[:, 0, :])
            nc.scalar.copy(out=ot[:, SQ_CHUNK:], in_=pp[:, 1, :])
            nc.sync.dma_start(out=out_d[b, ts(s, P), :], in_=ot)

        ot_box = {}

        def emit_outproj_piece(b, s, cc, tail=False):
            pp = pj.tile([P, SQ_CHUNK], F32, tag="pj", name=f"pp{b}_{s}_{cc}")
            nc.tensor.matmul(
                pp,
                lhsT=oT_sb[:, b, ts(s, P)],
                rhs=owT_sb[:, ts(cc, SQ_CHUNK)],
                start=True,
                stop=True,
            )
            if cc == 0:
                ot_box[(b, s)] = outsb.tile([P, D], MM_DT, tag="ot",
                                            name=f"ot{b}_{s}")
            ot = ot_box[(b, s)]
            if tail and cc == 0:
                nc.scalar.copy(out=ot[:, ts(cc, SQ_CHUNK)], in_=pp)
            else:
                nc.vector.tensor_copy(out=ot[:, ts(cc, SQ_CHUNK)], in_=pp)
            if cc == 1:
                nc.sync.dma_start(out=out_d[b, ts(s, P), :], in_=ot)
                del ot_box[(b, s)]

        def emit_flush(b, i, po_pack, tail=False):
            # po_pack[:, si, h, 0:64] = unnormalized o (token-major),
            # [.., 64] = softmax denominator. Normalize with a per-partition
            # scalar, transpose back to feature-major via the PE.
            for si, s in enumerate(range(4 * i, 4 * i + 4)):
                oTok = outsb.tile([P, FPC], F32, tag="otok",
                                  name=f"otok{b}{i}{si}")
                for h in range(HPC):
                    rc = small.tile([P, 1], F32, tag="rcp",
                                    name=f"rc{b}{i}{si}{h}")
                    nc.vector.reciprocal(
                        out=rc, in_=po_pack[:, si, h, HD:HD + 1])
                    nc.vector.tensor_scalar_mul(
                        out=oTok[:, h * HD:(h + 1) * HD],
                        in0=po_pack[:, si, h, 0:HD],
                        scalar1=rc,
                    )
                tp = pj.tile([P, P], F32, tag="pj", name=f"tp{b}{i}{si}")
                nc.tensor.transpose(tp, in_=oTok, identity=ident)
                nc.vector.tensor_copy(out=oT_sb[:, b, ts(s, P)], in_=tp)
                if tail and i == 0:
                    emit_outproj_tail(b, s)
                else:
                    for cc in range(2):
                        bg.append(lambda s=s, cc=cc:
                                  emit_outproj_piece(b, s, cc, tail))

        def att_chunk(b, i, bg_per_j):
            sq = ts(i, SQ_CHUNK)
            po_pack = po.tile([P, NSQ, HPC, HD + 1], F32, tag="po",
                              name=f"po{b}_{i}")
            # 8 accumulators share 2 PSUM banks; matmul start=True zeroes a
            # whole 2KB bank region, so pre-zero once and accumulate only.
            nc.vector.memset(po_pack, 0.0)
            jmax = 4 * i + 3
            for j in range(jmax + 1):
                # Columns < 128k of diagonal blocks are fully masked;
                # skip them in QK, exp and PV.
                k = j - 4 * i
                col0 = P * k if k > 0 else 0
                ps = mp.tile([P, HPC, SQ_CHUNK], F32, tag="mp",
                             name=f"ps{b}{i}{j}")
                # two heads' QK in adjacent PE row-tiles
                for h in range(HPC):
                    hs = h * HD
                    nc.tensor.matmul(
                        ps[:, h, col0:],
                        lhsT=kT_sb[hs:hs + HD, b, ts(j, P)],
                        rhs=qT_sb[hs:hs + HD, b,
                                  i * SQ_CHUNK + col0:(i + 1) * SQ_CHUNK],
                        start=True,
                        stop=True,
                    )
                pt = ptile.tile([P, HPC, SQ_CHUNK], MM_DT, tag="pt",
                                name=f"pt{b}{i}{j}")
                nc.scalar.activation(
                    out=pt[:, :, col0:], in_=ps[:, :, col0:],
                    func=mybir.ActivationFunctionType.Exp,
                    scale=EXP_SCALE,
                )
                if k >= 0:
                    # multiplicative 0/1 causal mask on the boundary s-block
                    nc.gpsimd.tensor_tensor(
                        out=pt[:, :, col0:col0 + P],
                        in0=pt[:, :, col0:col0 + P],
                        in1=maskm_sb[:, None, 0:P].to_broadcast([P, HPC, P]),
                        op=mybir.AluOpType.mult,
                    )
                drain_bg(bg_per_j)
                for h in range(HPC):
                    for si in range(NSQ):
                        if j <= 4 * i + si:
                            nc.tensor.matmul(
                                po_pack[:, si, h, :],
                                lhsT=pt[:, h, si * P:(si + 1) * P],
                                rhs=v_aug[:, b, h, j, :],
                                start=False,
                                stop=(j == 4 * i + si),
                                skip_group_check=True,
                            )
            emit_flush(b, i, po_pack, tail=(b == 1 and i <= 1))

        # ---- emission schedule ----
        # proj(b0,c0) upfront; att(b0,i) interleaves proj chunks of b0
        # then b1; att(b1,i) interleaves remaining proj + outproj(b0);
        # tail: outproj(b1).
        queue_proj_chunk(0, 0, split=True)
        drain_bg(1)  # xt(0,0) load first
        if emit_consts is not None:
            emit_consts()
        drain_bg(6)  # rest of chunk 0 (att(0,0) needs it)
        for cn in range(1, NSQ):
            queue_proj_chunk(0, cn)
        for cn in range(NSQ):
            queue_proj_chunk(1, cn)

        # j-steps per att chunk: 4,8,12,16 (x2 batches) = 80 total.
        # bg items: 7 per proj chunk x 7 remaining = 49, + 8 outproj
        # pieces per flushed chunk (queued by emit_flush as we go).
        for i in range(NSQ):
            att_chunk(0, i, bg_per_j=2)
        b1_pace = {3: 1, 2: 1, 1: 1, 0: 2}
        for i in range(NSQ - 1, -1, -1):
            att_chunk(1, i, bg_per_j=b1_pace[i])
        drain_bg(len(bg))


def get_module(repeat=1):
    key = ("nc", repeat)
    if key not in _module_cache:
        m = _build_module(repeat=repeat)
        m.compile()
        _module_cache[key] = m
    return _module_cache[key]


def make_in_maps(x, qw, qb, kw, kb, vw, vb, ow):
    import ml_dtypes
    mmdt = np.dtype(np.float16)
    fp8dt = mybir.dt.np(FP8)
    xT32 = np.ascontiguousarray(x.transpose(0, 2, 1))  # [B, D, S]
    xT = xT32.astype(mmdt)
    tp = np.arange(P, dtype=np.int64)[:, None]
    f1 = np.arange(P, dtype=np.int64)[None, :]
    f2 = np.arange(2 * P, dtype=np.int64)[None, :]
    t128 = (tp <= f1).astype(mmdt)
    t256 = (tp <= f2 - P).astype(mmdt)
    maskm = np.concatenate([t128, t256], axis=1)
    in_maps = []
    for c in range(NCORES):
        sl = slice(c * FPC, (c + 1) * FPC)
        wT = np.concatenate(
            [qw[sl, :].T, kw[sl, :].T, vw[sl, :].T], axis=1)
        qb_eff = qb[sl] * (W8_SCALE if FP8_QK else 1.0)
        m = {
            "xT": xT,
            "wT": np.ascontiguousarray(wT).astype(mmdt),
            "qb": np.ascontiguousarray(qb_eff.reshape(FPC, 1)).astype(np.float32),
            "owT": np.ascontiguousarray(ow[:, sl].T).astype(mmdt),
            "maskm": maskm,
        }
        if FP8_QK:
            m["x8"] = xT32.astype(fp8dt)
            w8 = np.concatenate(
                [qw[sl, :].T, kw[sl, :].T], axis=1) * W8_SCALE
            m["w8"] = np.ascontiguousarray(w8).astype(fp8dt)
        in_maps.append(m)
    return in_maps


def kernel(x, qw, qb, kw, kb, vw, vb, ow, ob, _trace=False):
    x = np.asarray(x, dtype=np.float32)
    qw = np.asarray(qw, dtype=np.float32)
    qb = np.asarray(qb, dtype=np.float32)
    kw = np.asarray(kw, dtype=np.float32)
    kb = np.asarray(kb, dtype=np.float32)
    vw = np.asarray(vw, dtype=np.float32)
    vb = np.asarray(vb, dtype=np.float32)
    ow = np.asarray(ow, dtype=np.float32)
    ob = np.asarray(ob, dtype=np.float32)

    nc = get_module()
    in_maps = make_in_maps(x, qw, qb, kw, kb, vw, vb, ow)
    res = run_bass_kernel_spmd(
        nc, in_maps, core_ids=list(range(NCORES)), trace=_trace
    )
    acc = np.zeros((B, S, D), dtype=np.float64)
    for r in res.results:
        acc += r["out"].astype(np.float64)
    # host-side bias folding: ob' = ob + ow @ vb (see module docstring)
    ob_eff = ob.astype(np.float64) + ow.astype(np.float64) @ vb.astype(np.float64)
    out = (acc + ob_eff).astype(np.float32)
    if _trace:
        kernel.last_results = res
    return out


# revision 36
# speedup vs baseline: 1.5096x; 1.5096x over previous
"""Head-sharded (tensor-parallel) causal attention block for 8 NeuronCores.

Model: B=2, S=2048, D=1024, H=16 heads (HD=64). Each core owns 2 heads
(128 features) of the QKV projections and attention, computes a partial
output projection (o_shard @ ow_shard) in fp32, and the host sums the 8
partials and adds the output bias.

Bias algebra (host-side folding):
  - k bias: softmax over keys is invariant to per-query constants;
    (q+qb)·(k+kb) = (q+qb)·k + const(q) -> kb dropped entirely.
  - v bias: softmax rows sum to 1, so o = sum p (v+vb) = sum p v + vb;
    after the output projection that is a constant vector ow @ vb folded
    into the output bias on the host. -> vb dropped entirely.
  Only qb is applied on-device (fused into the q PSUM->SBUF copy).

Per-core kernel phases (software-pipelined; attention chunk i only needs
projection chunks <= i, so proj/outproj matmuls are interleaved into the
attention j-loops via a background-work queue):
  1. Projections from xT chunks: qT/kT [feat 128, seq] accumulate 8
     D-blocks in PSUM (q: bias fused in the DVE copy; k: plain copy on
     Pool). V is computed directly token-major: V[t 128, feat 128] per
     t-block (lhsT = xT block, rhs = vwT block), written into
     v_aug[t, b, j, 0:64 | 65:129] with column 64 = 1.0 (the ones column
     makes the PV matmul also produce the softmax denominator).
  2. Attention per (batch, 512-query chunk): scores transposed
     sT[t,sq] = K@Q.T, two heads in the two PSUM banks of one tile; one
     exp on ScalarE covers both banks (scale=1/8 folded in). Causal mask
     applied multiplicatively (0/1 fp16) AFTER exp on the Pool engine,
     only on the boundary 128-col strip of diagonal t-blocks. PV is
     token-major: po_pack[s 128, si, h, 65] accumulates pt_block.T @
     v_aug per s-block with exact causal bounds (j <= 4i+si); the packed
     accumulators share banks, so the tile is pre-zeroed and matmuls use
     start=False (PSUM start=True zeroes whole 2KB bank regions).
  3. Normalize: per-partition reciprocal + tensor_scalar_mul (DVE), then
     a PE transpose (fp32 identity) restores feature-major oT_sb.
  4. Output projection: partial[sq,1024] in two 512-wide fp32 PSUM
     tiles; PSUM->SBUF cast copies alternate between DVE and Pool, then
     one fp16 DMA per half-row-block.

Matmul inputs are fp16 (full-rate on the PE).
"""

import numpy as np

import concourse.bass as bass
import concourse.mybir as mybir
import concourse.tile as tile
from concourse import bacc
from concourse.bass import ts
from concourse.bass_utils import run_bass_kernel_spmd
from concourse.masks import make_identity

B, S, D, H = 2, 2048, 1024, 16
HD = D // H            # 64 head dim
NCORES = 8
FPC = D // NCORES      # 128 features per core
HPC = FPC // HD        # 2 heads per core
P = 128
SQ_CHUNK = 512         # query chunk (matmul free dim)
NSQ = S // SQ_CHUNK    # 4
NTB = S // P           # 16 t-blocks
DBLK = D // P          # 8 contraction blocks for projections

F32 = mybir.dt.float32
MM_DT = mybir.dt.float16
FP8 = mybir.dt.float8e4
_NP_MM = "float16"
FP8_QK = False         # q/k projections in fp8e4 DoubleRow (weights x64)
W8_SCALE = 64.0
EXP_SCALE = 0.125 / (W8_SCALE * W8_SCALE) if FP8_QK else 0.125

_module_cache = {}


def _build_module(repeat=1):
    nc = bacc.Bacc("TRN2", target_bir_lowering=False, debug=False)

    xT_d = nc.dram_tensor("xT", [B, D, S], MM_DT, kind="ExternalInput").ap()
    wT_d = nc.dram_tensor("wT", [D, 3 * FPC], MM_DT, kind="ExternalInput").ap()
    if FP8_QK:
        x8_d = nc.dram_tensor("x8", [B, D, S], FP8, kind="ExternalInput").ap()
        w8_d = nc.dram_tensor("w8", [D, 2 * FPC], FP8, kind="ExternalInput").ap()
    qb_d = nc.dram_tensor("qb", [FPC, 1], F32, kind="ExternalInput").ap()
    owT_d = nc.dram_tensor("owT", [FPC, D], MM_DT, kind="ExternalInput").ap()
    maskm_d = nc.dram_tensor("maskm", [P, 3 * P], MM_DT, kind="ExternalInput").ap()
    out_d = nc.dram_tensor("out", [B, S, D], MM_DT, kind="ExternalOutput").ap()

    # [B, D, S] with D split into 8 blocks of 128 partitions
    xT_r = xT_d.rearrange("b (o p) s -> b p o s", p=P)
    x8_r = x8_d.rearrange("b (o p) s -> b p o s", p=P) if FP8_QK else None

    with tile.TileContext(nc) as tc:
        with (
            tc.tile_pool(name="singles", bufs=1) as singles,
            tc.tile_pool(name="xin", bufs=4) as xin,
            tc.tile_pool(name="ptile", bufs=6) as ptile,
            tc.tile_pool(name="small", bufs=8) as small,
        ):
            # --- constants / persistent tensors ---
            wT_sb = singles.tile([P, DBLK, 3 * FPC], MM_DT, tag="w")
            wT_r = wT_d.rearrange("(o p) m -> p o m", p=P)
            nc.sync.dma_start(out=wT_sb[:, :, 0:FPC], in_=wT_r[:, :, 0:FPC])
            qwT_sb = wT_sb[:, :, 0:FPC]
            kwT_sb = wT_sb[:, :, FPC:2 * FPC]
            vwT_sb = wT_sb[:, :, 2 * FPC:3 * FPC]
            if FP8_QK:
                w8_sb = singles.tile([P, DBLK, 2 * FPC], FP8, tag="w8")
                nc.sync.dma_start(
                    out=w8_sb, in_=w8_d.rearrange("(o p) m -> p o m", p=P))
            else:
                w8_sb = None
            qb_sb = singles.tile([FPC, 1], F32, tag="qb")
            owT_sb = singles.tile([FPC, D], MM_DT, tag="ow")
            maskm_sb = singles.tile([P, 3 * P], MM_DT, tag="maskm")

            def emit_consts():
                nc.scalar.dma_start(out=qb_sb, in_=qb_d)
                nc.scalar.dma_start(out=wT_sb[:, :, FPC:], in_=wT_r[:, :, FPC:])
                nc.scalar.dma_start(out=owT_sb, in_=owT_d)
                nc.scalar.dma_start(out=maskm_sb, in_=maskm_d)
            ident = singles.tile([P, P], F32, tag="ident")
            make_identity(nc, ident)

            qT_sb = singles.tile([P, B, S], MM_DT, tag="qT")
            kT_sb = singles.tile([P, B, S], MM_DT, tag="kT")
            oT_sb = singles.tile([P, B, S], MM_DT, tag="oT")
            # V_aug[t, b, h, tblk, 0:64] = v feats; [.., 64] = 1.0 (the
            # ones column makes the PV matmul also emit the denominator
            # in po row 64 for both heads).
            v_aug = singles.tile([P, B, HPC, NTB, HD + 1], MM_DT, tag="vaug")
            ones_sb = singles.tile([P, 1], F32, tag="ones")
            nc.vector.memset(ones_sb, 1.0)
            nc.vector.tensor_copy(
                out=v_aug[:, :, :, :, HD],
                in_=ones_sb[:, 0][:, None, None, None].to_broadcast([P, B, HPC, NTB]),
            )

            # ---------- repetitions (>1 only for HW timing calibration) ---
            for _rep in range(repeat):
                _emit_body(nc, tc, locals(), emit_consts if _rep == 0 else None)

    return nc


def _emit_body(nc, tc, env, emit_consts=None):
    g = type("G", (), env)
    xin, ptile, small = g.xin, g.ptile, g.small
    qwT_sb, kwT_sb, vwT_sb, w8_sb = g.qwT_sb, g.kwT_sb, g.vwT_sb, g.w8_sb
    x8_r = g.x8_r
    qb_sb, owT_sb, maskm_sb = g.qb_sb, g.owT_sb, g.maskm_sb
    qT_sb, kT_sb, oT_sb, v_aug = g.qT_sb, g.kT_sb, g.oT_sb, g.v_aug
    ident = g.ident
    xT_r, out_d = g.xT_r, g.out_d

    # PSUM budget (8 banks): pj 2x[P,512] (proj accum, then outproj) = 2,
    # mp 2x[P,2,512] (QK scores) = 4, po 2x[65,512] (PV accum) = 2.
    with (
        tc.tile_pool(name="outsb", bufs=6) as outsb,
        tc.tile_pool(name="pj", bufs=2, space="PSUM") as pj,
        tc.tile_pool(name="mp", bufs=2, space="PSUM") as mp,
        tc.tile_pool(name="po", bufs=1, space="PSUM") as po,
    ):
        # ---- background work queue: each item emits a small batch of
        # independent PE work (proj piece / outproj piece); popped between
        # attention j-steps to fill PE stalls while ScalarE does exp.
        bg = []

        def drain_bg(n):
            for _ in range(n):
                if bg:
                    bg.pop(0)()

        def emit_xt_load(b, cn, split=False):
            x8t = None
            if FP8_QK:
                x8t = xin.tile([P, DBLK, SQ_CHUNK], FP8, tag="x8t",
                               name=f"x8t{b}{cn}")
                nc.sync.dma_start(
                    out=x8t, in_=x8_r[b, :, :, ts(cn, SQ_CHUNK)],
                )
            xt = xin.tile([P, DBLK, SQ_CHUNK], MM_DT, tag="xt",
                          name=f"xt{b}{cn}")
            if split:
                nc.scalar.dma_start(
                    out=xt[:, 0:2, :],
                    in_=xT_r[b, :, 0:2, ts(cn, SQ_CHUNK)],
                )
                for q in range(2, DBLK, 2):
                    nc.sync.dma_start(
                        out=xt[:, q:q + 2, :],
                        in_=xT_r[b, :, q:q + 2, ts(cn, SQ_CHUNK)],
                    )
            else:
                nc.sync.dma_start(
                    out=xt, in_=xT_r[b, :, :, ts(cn, SQ_CHUNK)],
                )
            return xt, x8t

        def emit_qk_piece(b, cn, xtpair, kind):
            xt, x8t = xtpair
            ps = pj.tile([P, SQ_CHUNK], F32, tag="pj", name=f"prj{b}{cn}{kind}")
            if FP8_QK:
                f0 = 0 if kind == "q" else FPC
                for op in range(DBLK // 2):
                    nc.tensor.matmul(
                        ps,
                        lhsT=w8_sb[:, 2 * op:2 * op + 2, f0:f0 + FPC],
                        rhs=x8t[:, 2 * op:2 * op + 2, :],
                        start=(op == 0),
                        stop=(op == DBLK // 2 - 1),
                        perf_mode=mybir.MatmulPerfMode.DoubleRow,
                    )
            else:
                wT_sb = qwT_sb if kind == "q" else kwT_sb
                for o in range(DBLK):
                    nc.tensor.matmul(
                        ps,
                        lhsT=wT_sb[:, o, :],
                        rhs=xt[:, o, :],
                        start=(o == 0),
                        stop=(o == DBLK - 1),
                    )
            if kind == "q":
                nc.vector.tensor_scalar_add(
                    out=qT_sb[:, b, ts(cn, SQ_CHUNK)], in0=ps, scalar1=qb_sb,
                )
            else:
                nc.vector.tensor_copy(
                    out=kT_sb[:, b, ts(cn, SQ_CHUNK)], in_=ps,
                )

        def emit_v_piece(b, cn, xtpair, jj):
            xt = xtpair[0]
            # token-major V for t-block j: [t 128, feat 128]
            j = 4 * cn + jj
            vps = pj.tile([P, FPC], F32, tag="pj", name=f"vp{b}{j}")
            toff = jj * P
            for o in range(DBLK):
                nc.tensor.matmul(
                    vps,
                    lhsT=xt[:, o, toff:toff + P],
                    rhs=vwT_sb[:, o, :],
                    start=(o == 0),
                    stop=(o == DBLK - 1),
                )
            nc.vector.tensor_copy(out=v_aug[:, b, 0, j, 0:HD], in_=vps[:, 0:HD])
            nc.vector.tensor_copy(
                out=v_aug[:, b, 1, j, 0:HD], in_=vps[:, HD:FPC],
            )

        def queue_proj_chunk(b, cn, split=False):
            xt_box = []

            def load():
                xt_box.append(emit_xt_load(b, cn, split))

            bg.append(load)
            bg.append(lambda: emit_qk_piece(b, cn, xt_box[0], "q"))
            bg.append(lambda: emit_qk_piece(b, cn, xt_box[0], "k"))
            for jj in range(4):
                bg.append(lambda jj=jj: emit_v_piece(b, cn, xt_box[0], jj))

        def emit_outproj_tail(b, s):
            pp = mp.tile([P, HPC, SQ_CHUNK], F32, tag="mp", name=f"ppt{b}_{s}")
            for cc in range(2):
                nc.tensor.matmul(
                    pp[:, cc, :],
                    lhsT=oT_sb[:, b, ts(s, P)],
                    rhs=owT_sb[:, ts(cc, SQ_CHUNK)],
                    start=True,
                    stop=True,
                )
            ot = outsb.tile([P, D], MM_DT, tag="ot", name=f"ott{b}_{s}")
            nc.vector.tensor_copy(out=ot[:, 0:SQ_CHUNK], in_=pp[:, 0, :])
            nc.scalar.copy(out=ot[:, SQ_CHUNK:], in_=pp[:, 1, :])
            nc.sync.dma_start(out=out_d[b, ts(s, P), :], in_=ot)

        ot_box = {}

        def emit_outproj_piece(b, s, cc, tail=False):
            pp = pj.tile([P, SQ_CHUNK], F32, tag="pj", name=f"pp{b}_{s}_{cc}")
            nc.tensor.matmul(
                pp,
                lhsT=oT_sb[:, b, ts(s, P)],
                rhs=owT_sb[:, ts(cc, SQ_CHUNK)],
                start=True,
                stop=True,
            )
            if cc == 0:
                ot_box[(b, s)] = outsb.tile([P, D], MM_DT, tag="ot",
                                            name=f"ot{b}_{s}")
            ot = ot_box[(b, s)]
            if tail and cc == 0:
                nc.scalar.copy(out=ot[:, ts(cc, SQ_CHUNK)], in_=pp)
            else:
                nc.vector.tensor_copy(out=ot[:, ts(cc, SQ_CHUNK)], in_=pp)
            if cc == 1:
                nc.sync.dma_start(out=out_d[b, ts(s, P), :], in_=ot)
                del ot_box[(b, s)]

        def emit_flush(b, i, po_pack, tail=False):
            # po_pack[:, si, h, 0:64] = unnormalized o (token-major),
            # [.., 64] = softmax denominator. Normalize with a per-partition
            # scalar, transpose back to feature-major via the PE.
            for si, s in enumerate(range(4 * i, 4 * i + 4)):
                oTok = outsb.tile([P, FPC], F32, tag="otok",
                                  name=f"otok{b}{i}{si}")
                for h in range(HPC):
                    rc = small.tile([P, 1], F32, tag="rcp",
                                    name=f"rc{b}{i}{si}{h}")
                    nc.vector.reciprocal(
                        out=rc, in_=po_pack[:, si, h, HD:HD + 1])
                    nc.vector.tensor_scalar_mul(
                        out=oTok[:, h * HD:(h + 1) * HD],
                        in0=po_pack[:, si, h, 0:HD],
                        scalar1=rc,
                    )
                tp = pj.tile([P, P], F32, tag="pj", name=f"tp{b}{i}{si}")
                nc.tensor.transpose(tp, in_=oTok, identity=ident)
                nc.vector.tensor_copy(out=oT_sb[:, b, ts(s, P)], in_=tp)
                if tail and i == 0:
                    emit_outproj_tail(b, s)
                else:
                    for cc in range(2):
                        bg.append(lambda s=s, cc=cc:
                                  emit_outproj_piece(b, s, cc, tail))

        def att_chunk(b, i, bg_per_j):
            sq = ts(i, SQ_CHUNK)
            po_pack = po.tile([P, NSQ, HPC, HD + 1], F32, tag="po",
                              name=f"po{b}_{i}")
            # 8 accumulators share 2 PSUM banks; matmul start=True zeroes a
            # whole 2KB bank region, so pre-zero once and accumulate only.
            nc.vector.memset(po_pack, 0.0)
            jmax = 4 * i + 3
            for j in range(jmax + 1):
                # Columns < 128k of diagonal blocks are fully masked;
                # skip them in QK, exp and PV.
                k = j - 4 * i
                col0 = P * k if k > 0 else 0
                ps = mp.tile([P, HPC, SQ_CHUNK], F32, tag="mp",
                             name=f"ps{b}{i}{j}")
                # two heads' QK in adjacent PE row-tiles
                for h in range(HPC):
                    hs = h * HD
                    nc.tensor.matmul(
                        ps[:, h, col0:],
                        lhsT=kT_sb[hs:hs + HD, b, ts(j, P)],
                        rhs=qT_sb[hs:hs + HD, b,
                                  i * SQ_CHUNK + col0:(i + 1) * SQ_CHUNK],
                        start=True,
                        stop=True,
                    )
                pt = ptile.tile([P, HPC, SQ_CHUNK], MM_DT, tag="pt",
                                name=f"pt{b}{i}{j}")
                nc.scalar.activation(
                    out=pt[:, :, col0:], in_=ps[:, :, col0:],
                    func=mybir.ActivationFunctionType.Exp,
                    scale=EXP_SCALE,
                )
                if k >= 0:
                    # multiplicative 0/1 causal mask on the boundary s-block
                    nc.gpsimd.tensor_tensor(
                        out=pt[:, :, col0:col0 + P],
                        in0=pt[:, :, col0:col0 + P],
                        in1=maskm_sb[:, None, 0:P].to_broadcast([P, HPC, P]),
                        op=mybir.AluOpType.mult,
                    )
                drain_bg(bg_per_j)
                for h in range(HPC):
                    for si in range(NSQ):
                        if j <= 4 * i + si:
                            nc.tensor.matmul(
                                po_pack[:, si, h, :],
                                lhsT=pt[:, h, si * P:(si + 1) * P],
                                rhs=v_aug[:, b, h, j, :],
                                start=False,
                                stop=(j == 4 * i + si),
                                skip_group_check=True,
                            )
            emit_flush(b, i, po_pack, tail=(b == 1 and i <= 1))

        # ---- emission schedule ----
        # proj(b0,c0) upfront; att(b0,i) interleaves proj chunks of b0
        # then b1; att(b1,i) interleaves remaining proj + outproj(b0);
        # tail: outproj(b1).
        queue_proj_chunk(0, 0, split=True)
        drain_bg(1)  # xt(0,0) load first
        if emit_consts is not None:
            emit_consts()
        drain_bg(6)  # rest of chunk 0 (att(0,0) needs it)
        for cn in range(1, NSQ):
            queue_proj_chunk(0, cn)
        for cn in range(NSQ):
            queue_proj_chunk(1, cn)

        # j-steps per att chunk: 4,8,12,16 (x2 batches) = 80 total.
        # bg items: 7 per proj chunk x 7 remaining = 49, + 8 outproj
        # pieces per flushed chunk (queued by emit_flush as we go).
        for i in range(NSQ):
            att_chunk(0, i, bg_per_j=2)
        b1_pace = {3: 1, 2: 1, 1: 2, 0: 2}
        for i in range(NSQ - 1, -1, -1):
            att_chunk(1, i, bg_per_j=b1_pace[i])
        drain_bg(len(bg))


def get_module(repeat=1):
    key = ("nc", repeat)
    if key not in _module_cache:
        m = _build_module(repeat=repeat)
        m.compile()
        _module_cache[key] = m
    return _module_cache[key]


def make_in_maps(x, qw, qb, kw, kb, vw, vb, ow):
    import ml_dtypes
    mmdt = np.dtype(np.float16)
    fp8dt = mybir.dt.np(FP8)
    xT32 = np.ascontiguousarray(x.transpose(0, 2, 1))  # [B, D, S]
    xT = xT32.astype(mmdt)
    tp = np.arange(P, dtype=np.int64)[:, None]
    f1 = np.arange(P, dtype=np.int64)[None, :]
    f2 = np.arange(2 * P, dtype=np.int64)[None, :]
    t128 = (tp <= f1).astype(mmdt)
    t256 = (tp <= f2 - P).astype(mmdt)
    maskm = np.concatenate([t128, t256], axis=1)
    in_maps = []
    for c in range(NCORES):
        sl = slice(c * FPC, (c + 1) * FPC)
        wT = np.concatenate(
            [qw[sl, :].T, kw[sl, :].T, vw[sl, :].T], axis=1)
        qb_eff = qb[sl] * (W8_SCALE if FP8_QK else 1.0)
        m = {
            "xT": xT,
            "wT": np.ascontiguousarray(wT).astype(mmdt),
            "qb": np.ascontiguousarray(qb_eff.reshape(FPC, 1)).astype(np.float32),
            "owT": np.ascontiguousarray(ow[:, sl].T).astype(mmdt),
            "maskm": maskm,
        }
        if FP8_QK:
            m["x8"] = xT32.astype(fp8dt)
            w8 = np.concatenate(
                [qw[sl, :].T, kw[sl, :].T], axis=1) * W8_SCALE
            m["w8"] = np.ascontiguousarray(w8).astype(fp8dt)
        in_maps.append(m)
    return in_maps


def kernel(x, qw, qb, kw, kb, vw, vb, ow, ob, _trace=False):
    x = np.asarray(x, dtype=np.float32)
    qw = np.asarray(qw, dtype=np.float32)
    qb = np.asarray(qb, dtype=np.float32)
    kw = np.asarray(kw, dtype=np.float32)
    kb = np.asarray(kb, dtype=np.float32)
    vw = np.asarray(vw, dtype=np.float32)
    vb = np.asarray(vb, dtype=np.float32)
    ow = np.asarray(ow, dtype=np.float32)
    ob = np.asarray(ob, dtype=np.float32)

    nc = get_module()
    in_maps = make_in_maps(x, qw, qb, kw, kb, vw, vb, ow)
    res = run_bass_kernel_spmd(
        nc, in_maps, core_ids=list(range(NCORES)), trace=_trace
    )
    acc = np.zeros((B, S, D), dtype=np.float64)
    for r in res.results:
        acc += r["out"].astype(np.float64)
    # host-side bias folding: ob' = ob + ow @ vb (see module docstring)
    ob_eff = ob.astype(np.float64) + ow.astype(np.float64) @ vb.astype(np.float64)
    out = (acc + ob_eff).astype(np.float32)
    if _trace:
        kernel.last_results = res
    return out


# revision 46
# speedup vs baseline: 2.5581x; 1.6945x over previous
"""Head-sharded (tensor-parallel) causal attention block for 8 NeuronCores.

Model: B=2, S=2048, D=1024, H=16 heads (HD=64). Each core owns 2 heads
(128 features) of the QKV projections and attention, computes a partial
output projection (o_shard @ ow_shard) in fp32, and the host sums the 8
partials and adds the output bias.

Bias algebra (host-side folding):
  - k bias: softmax over keys is invariant to per-query constants;
    (q+qb)·(k+kb) = (q+qb)·k + const(q) -> kb dropped entirely.
  - v bias: softmax rows sum to 1, so o = sum p (v+vb) = sum p v + vb;
    after the output projection that is a constant vector ow @ vb folded
    into the output bias on the host. -> vb dropped entirely.
  Only qb is applied on-device (fused into the q PSUM->SBUF copy).

Per-core kernel phases (software-pipelined; attention chunk i only needs
projection chunks <= i, so proj/outproj matmuls are interleaved into the
attention j-loops via a background-work queue):
  1. Projections from xT chunks: qT/kT [feat 128, seq] accumulate 8
     D-blocks in PSUM (q: bias fused in the DVE copy; k: plain copy on
     Pool). V is computed directly token-major: V[t 128, feat 128] per
     t-block (lhsT = xT block, rhs = vwT block), written into
     v_aug[t, b, j, 0:64 | 65:129] with column 64 = 1.0 (the ones column
     makes the PV matmul also produce the softmax denominator).
  2. Attention per (batch, 512-query chunk): scores transposed
     sT[t,sq] = K@Q.T, two heads in the two PSUM banks of one tile; one
     exp on ScalarE covers both banks (scale=1/8 folded in). Causal mask
     applied multiplicatively (0/1 fp16) AFTER exp on the Pool engine,
     only on the boundary 128-col strip of diagonal t-blocks. PV is
     token-major: po_pack[s 128, si, h, 65] accumulates pt_block.T @
     v_aug per s-block with exact causal bounds (j <= 4i+si); the packed
     accumulators share banks, so the tile is pre-zeroed and matmuls use
     start=False (PSUM start=True zeroes whole 2KB bank regions).
  3. Normalize: per-partition reciprocal + tensor_scalar_mul (DVE), then
     a PE transpose (fp32 identity) restores feature-major oT_sb.
  4. Output projection: partial[sq,1024] in two 512-wide fp32 PSUM
     tiles; PSUM->SBUF cast copies alternate between DVE and Pool, then
     one fp16 DMA per half-row-block.

Matmul inputs are fp16 (full-rate on the PE).
"""

import numpy as np

import concourse.bass as bass
import concourse.mybir as mybir
import concourse.tile as tile
from concourse import bacc
from concourse.bass import ts
from concourse.bass_utils import run_bass_kernel_spmd
from concourse.masks import make_identity

B, S, D, H = 2, 2048, 1024, 16
HD = D // H            # 64 head dim
NCORES = 8
FPC = D // NCORES      # 128 features per core
HPC = FPC // HD        # 2 heads per core
P = 128
SQ_CHUNK = 512         # query chunk (matmul free dim)
NSQ = S // SQ_CHUNK    # 4
NTB = S // P           # 16 t-blocks
DBLK = D // P          # 8 contraction blocks for projections

F32 = mybir.dt.float32
MM_DT = mybir.dt.float16
FP8 = mybir.dt.float8e4
_NP_MM = "float16"
FP8_QK = False         # q/k projections in fp8e4 DoubleRow (weights x64)
W8_SCALE = 64.0
EXP_SCALE = 0.125 / (W8_SCALE * W8_SCALE) if FP8_QK else 0.125

_module_cache = {}


def _build_module(repeat=1):
    nc = bacc.Bacc("TRN2", target_bir_lowering=False, debug=False)

    xT_d = nc.dram_tensor("xT", [B, D, S], MM_DT, kind="ExternalInput").ap()
    wT_d = nc.dram_tensor("wT", [D, 3 * FPC], MM_DT, kind="ExternalInput").ap()
    if FP8_QK:
        x8_d = nc.dram_tensor("x8", [B, D, S], FP8, kind="ExternalInput").ap()
        w8_d = nc.dram_tensor("w8", [D, 2 * FPC], FP8, kind="ExternalInput").ap()
    qb_d = nc.dram_tensor("qb", [FPC, 1], F32, kind="ExternalInput").ap()
    owT_d = nc.dram_tensor("owT", [FPC, D], MM_DT, kind="ExternalInput").ap()
    maskm_d = nc.dram_tensor("maskm", [P, 3 * P], MM_DT, kind="ExternalInput").ap()
    out_d = nc.dram_tensor("out", [B, S, D], MM_DT, kind="ExternalOutput").ap()

    # [B, D, S] with D split into 8 blocks of 128 partitions
    xT_r = xT_d.rearrange("b (o p) s -> b p o s", p=P)
    x8_r = x8_d.rearrange("b (o p) s -> b p o s", p=P) if FP8_QK else None

    with tile.TileContext(nc) as tc:
        with (
            tc.tile_pool(name="singles", bufs=1) as singles,
            tc.tile_pool(name="xin", bufs=6) as xin,
            tc.tile_pool(name="ptile", bufs=12) as ptile,
            tc.tile_pool(name="small", bufs=10) as small,
        ):
            # --- constants / persistent tensors ---
            wT_sb = singles.tile([P, DBLK, 3 * FPC], MM_DT, tag="w")
            wT_r = wT_d.rearrange("(o p) m -> p o m", p=P)
            nc.sync.dma_start(out=wT_sb[:, :, 0:FPC], in_=wT_r[:, :, 0:FPC])
            qwT_sb = wT_sb[:, :, 0:FPC]
            kwT_sb = wT_sb[:, :, FPC:2 * FPC]
            vwT_sb = wT_sb[:, :, 2 * FPC:3 * FPC]
            if FP8_QK:
                w8_sb = singles.tile([P, DBLK, 2 * FPC], FP8, tag="w8")
                nc.sync.dma_start(
                    out=w8_sb, in_=w8_d.rearrange("(o p) m -> p o m", p=P))
            else:
                w8_sb = None
            qb_sb = singles.tile([FPC, 1], F32, tag="qb")
            owT_sb = singles.tile([FPC, D], MM_DT, tag="ow")
            maskm_sb = singles.tile([P, 3 * P], MM_DT, tag="maskm")

            def emit_consts():
                nc.scalar.dma_start(out=qb_sb, in_=qb_d)
                nc.scalar.dma_start(out=wT_sb[:, :, FPC:], in_=wT_r[:, :, FPC:])
                nc.scalar.dma_start(out=owT_sb, in_=owT_d)
                nc.scalar.dma_start(out=maskm_sb, in_=maskm_d)
            ident = singles.tile([P, P], F32, tag="ident")
            make_identity(nc, ident)

            qT_sb = singles.tile([P, B, S], MM_DT, tag="qT")
            kT_sb = singles.tile([P, B, S], MM_DT, tag="kT")
            oT_sb = singles.tile([P, B, S], MM_DT, tag="oT")
            # V_aug[t, b, h, tblk, 0:64] = v feats; [.., 64] = 1.0 (the
            # ones column makes the PV matmul also emit the denominator
            # in po row 64 for both heads).
            v_aug = singles.tile([P, B, HPC, NTB, HD + 1], MM_DT, tag="vaug")
            ones_sb = singles.tile([P, 1], F32, tag="ones")
            nc.vector.memset(ones_sb, 1.0)
            nc.vector.tensor_copy(
                out=v_aug[:, :, :, :, HD],
                in_=ones_sb[:, 0][:, None, None, None].to_broadcast([P, B, HPC, NTB]),
            )

            # ---------- repetitions (>1 only for HW timing calibration) ---
            for _rep in range(repeat):
                _emit_body(nc, tc, locals(), emit_consts if _rep == 0 else None)

    return nc


def _emit_body(nc, tc, env, emit_consts=None):
    g = type("G", (), env)
    xin, ptile, small = g.xin, g.ptile, g.small
    qwT_sb, kwT_sb, vwT_sb, w8_sb = g.qwT_sb, g.kwT_sb, g.vwT_sb, g.w8_sb
    x8_r = g.x8_r
    qb_sb, owT_sb, maskm_sb = g.qb_sb, g.owT_sb, g.maskm_sb
    qT_sb, kT_sb, oT_sb, v_aug = g.qT_sb, g.kT_sb, g.oT_sb, g.v_aug
    ident = g.ident
    xT_r, out_d = g.xT_r, g.out_d

    # PSUM budget (8 banks): pj 2x[P,512] (proj accum, then outproj) = 2,
    # mp 2x[P,2,512] (QK scores) = 4, po 2x[65,512] (PV accum) = 2.
    with (
        tc.tile_pool(name="outsb", bufs=8) as outsb,
        tc.tile_pool(name="pj", bufs=2, space="PSUM") as pj,
        tc.tile_pool(name="mp", bufs=2, space="PSUM") as mp,
        tc.tile_pool(name="po", bufs=1, space="PSUM") as po,
    ):
        # ---- background work queue: each item emits a small batch of
        # independent PE work (proj piece / outproj piece); popped between
        # attention j-steps to fill PE stalls while ScalarE does exp.
        bg = []

        def drain_bg(n):
            for _ in range(n):
                if bg:
                    bg.pop(0)()

        def emit_xt_load(b, cn, split=False):
            x8t = None
            if FP8_QK:
                x8t = xin.tile([P, DBLK, SQ_CHUNK], FP8, tag="x8t",
                               name=f"x8t{b}{cn}")
                nc.sync.dma_start(
                    out=x8t, in_=x8_r[b, :, :, ts(cn, SQ_CHUNK)],
                )
            xt = xin.tile([P, DBLK, SQ_CHUNK], MM_DT, tag="xt",
                          name=f"xt{b}{cn}")
            if split:
                nc.scalar.dma_start(
                    out=xt[:, 0:2, :],
                    in_=xT_r[b, :, 0:2, ts(cn, SQ_CHUNK)],
                )
                for q in range(2, DBLK, 2):
                    nc.sync.dma_start(
                        out=xt[:, q:q + 2, :],
                        in_=xT_r[b, :, q:q + 2, ts(cn, SQ_CHUNK)],
                    )
            else:
                nc.sync.dma_start(
                    out=xt, in_=xT_r[b, :, :, ts(cn, SQ_CHUNK)],
                )
            return xt, x8t

        def emit_qk_piece(b, cn, xtpair, kind):
            xt, x8t = xtpair
            ps = pj.tile([P, SQ_CHUNK], F32, tag="pj", name=f"prj{b}{cn}{kind}")
            if FP8_QK:
                f0 = 0 if kind == "q" else FPC
                for op in range(DBLK // 2):
                    nc.tensor.matmul(
                        ps,
                        lhsT=w8_sb[:, 2 * op:2 * op + 2, f0:f0 + FPC],
                        rhs=x8t[:, 2 * op:2 * op + 2, :],
                        start=(op == 0),
                        stop=(op == DBLK // 2 - 1),
                        perf_mode=mybir.MatmulPerfMode.DoubleRow,
                    )
            else:
                wT_sb = qwT_sb if kind == "q" else kwT_sb
                for o in range(DBLK):
                    nc.tensor.matmul(
                        ps,
                        lhsT=wT_sb[:, o, :],
                        rhs=xt[:, o, :],
                        start=(o == 0),
                        stop=(o == DBLK - 1),
                    )
            if kind == "q":
                nc.vector.tensor_scalar_add(
                    out=qT_sb[:, b, ts(cn, SQ_CHUNK)], in0=ps, scalar1=qb_sb,
                )
            else:
                nc.vector.tensor_copy(
                    out=kT_sb[:, b, ts(cn, SQ_CHUNK)], in_=ps,
                )

        def emit_v_piece(b, cn, xtpair, jj):
            xt = xtpair[0]
            # token-major V for t-block j: [t 128, feat 128]
            j = 4 * cn + jj
            vps = pj.tile([P, FPC], F32, tag="pj", name=f"vp{b}{j}")
            toff = jj * P
            for o in range(DBLK):
                nc.tensor.matmul(
                    vps,
                    lhsT=xt[:, o, toff:toff + P],
                    rhs=vwT_sb[:, o, :],
                    start=(o == 0),
                    stop=(o == DBLK - 1),
                )
            nc.vector.tensor_copy(out=v_aug[:, b, 0, j, 0:HD], in_=vps[:, 0:HD])
            nc.vector.tensor_copy(
                out=v_aug[:, b, 1, j, 0:HD], in_=vps[:, HD:FPC],
            )

        def queue_proj_chunk(b, cn, split=False):
            xt_box = []

            def load():
                xt_box.append(emit_xt_load(b, cn, split))

            bg.append(load)
            bg.append(lambda: emit_qk_piece(b, cn, xt_box[0], "q"))
            bg.append(lambda: emit_qk_piece(b, cn, xt_box[0], "k"))
            for jj in range(4):
                bg.append(lambda jj=jj: emit_v_piece(b, cn, xt_box[0], jj))

        def emit_outproj_tail(b, s):
            pp = mp.tile([P, HPC, SQ_CHUNK], F32, tag="mp", name=f"ppt{b}_{s}")
            for cc in range(2):
                nc.tensor.matmul(
                    pp[:, cc, :],
                    lhsT=oT_sb[:, b, ts(s, P)],
                    rhs=owT_sb[:, ts(cc, SQ_CHUNK)],
                    start=True,
                    stop=True,
                )
            ot = outsb.tile([P, D], MM_DT, tag="ot", name=f"ott{b}_{s}")
            nc.vector.tensor_copy(out=ot[:, 0:SQ_CHUNK], in_=pp[:, 0, :])
            nc.scalar.copy(out=ot[:, SQ_CHUNK:], in_=pp[:, 1, :])
            nc.sync.dma_start(out=out_d[b, ts(s, P), :], in_=ot)

        ot_box = {}

        def emit_outproj_piece(b, s, cc, tail=False):
            pp = pj.tile([P, SQ_CHUNK], F32, tag="pj", name=f"pp{b}_{s}_{cc}")
            nc.tensor.matmul(
                pp,
                lhsT=oT_sb[:, b, ts(s, P)],
                rhs=owT_sb[:, ts(cc, SQ_CHUNK)],
                start=True,
                stop=True,
            )
            if cc == 0:
                ot_box[(b, s)] = outsb.tile([P, D], MM_DT, tag="ot",
                                            name=f"ot{b}_{s}")
            ot = ot_box[(b, s)]
            if tail and cc == 0:
                nc.scalar.copy(out=ot[:, ts(cc, SQ_CHUNK)], in_=pp)
            else:
                nc.vector.tensor_copy(out=ot[:, ts(cc, SQ_CHUNK)], in_=pp)
            if cc == 1:
                nc.sync.dma_start(out=out_d[b, ts(s, P), :], in_=ot)
                del ot_box[(b, s)]

        def emit_flush(b, i, po_pack, tail=False):
            # po_pack[:, si, h, 0:64] = unnormalized o (token-major),
            # [.., 64] = softmax denominator. Normalize with a per-partition
            # scalar, transpose back to feature-major via the PE.
            for si, s in enumerate(range(4 * i, 4 * i + 4)):
                oTok = outsb.tile([P, FPC], F32, tag="otok",
                                  name=f"otok{b}{i}{si}")
                for h in range(HPC):
                    rc = small.tile([P, 1], F32, tag="rcp",
                                    name=f"rc{b}{i}{si}{h}")
                    nc.vector.reciprocal(
                        out=rc, in_=po_pack[:, si, h, HD:HD + 1])
                    nc.vector.tensor_scalar_mul(
                        out=oTok[:, h * HD:(h + 1) * HD],
                        in0=po_pack[:, si, h, 0:HD],
                        scalar1=rc,
                    )
                tp = pj.tile([P, P], F32, tag="pj", name=f"tp{b}{i}{si}")
                nc.tensor.transpose(tp, in_=oTok, identity=ident)
                nc.vector.tensor_copy(out=oT_sb[:, b, ts(s, P)], in_=tp)
                if tail and i == 0:
                    emit_outproj_tail(b, s)
                else:
                    for cc in range(2):
                        bg.append(lambda s=s, cc=cc:
                                  emit_outproj_piece(b, s, cc, tail))

        def att_chunk(b, i, bg_per_j):
            sq = ts(i, SQ_CHUNK)
            po_pack = po.tile([P, NSQ, HPC, HD + 1], F32, tag="po",
                              name=f"po{b}_{i}")
            # 8 accumulators share 2 PSUM banks; matmul start=True zeroes a
            # whole 2KB bank region, so pre-zero once and accumulate only.
            nc.vector.memset(po_pack, 0.0)
            jmax = 4 * i + 3
            for j in range(jmax + 1):
                # Columns < 128k of diagonal blocks are fully masked;
                # skip them in QK, exp and PV.
                k = j - 4 * i
                col0 = P * k if k > 0 else 0
                ps = mp.tile([P, HPC, SQ_CHUNK], F32, tag="mp",
                             name=f"ps{b}{i}{j}")
                # two heads' QK in adjacent PE row-tiles
                for h in range(HPC):
                    hs = h * HD
                    nc.tensor.matmul(
                        ps[:, h, col0:],
                        lhsT=kT_sb[hs:hs + HD, b, ts(j, P)],
                        rhs=qT_sb[hs:hs + HD, b,
                                  i * SQ_CHUNK + col0:(i + 1) * SQ_CHUNK],
                        start=True,
                        stop=True,
                    )
                pt = ptile.tile([P, HPC, SQ_CHUNK], MM_DT, tag="pt",
                                name=f"pt{b}{i}{j}")
                nc.scalar.activation(
                    out=pt[:, :, col0:], in_=ps[:, :, col0:],
                    func=mybir.ActivationFunctionType.Exp,
                    scale=EXP_SCALE,
                )
                if k >= 0:
                    # multiplicative 0/1 causal mask on the boundary s-block
                    nc.gpsimd.tensor_tensor(
                        out=pt[:, :, col0:col0 + P],
                        in0=pt[:, :, col0:col0 + P],
                        in1=maskm_sb[:, None, 0:P].to_broadcast([P, HPC, P]),
                        op=mybir.AluOpType.mult,
                    )
                drain_bg(bg_per_j)
                for h in range(HPC):
                    for si in range(NSQ):
                        if j <= 4 * i + si:
                            nc.tensor.matmul(
                                po_pack[:, si, h, :],
                                lhsT=pt[:, h, si * P:(si + 1) * P],
                                rhs=v_aug[:, b, h, j, :],
                                start=False,
                                stop=(j == 4 * i + si),
                                skip_group_check=True,
                            )
            emit_flush(b, i, po_pack, tail=(b == 1 and i <= 1))

        # ---- emission schedule ----
        # proj(b0,c0) upfront; att(b0,i) interleaves proj chunks of b0
        # then b1; att(b1,i) interleaves remaining proj + outproj(b0);
        # tail: outproj(b1).
        queue_proj_chunk(0, 0, split=True)
        drain_bg(1)  # xt(0,0) load first
        if emit_consts is not None:
            emit_consts()
        drain_bg(6)  # rest of chunk 0 (att(0,0) needs it)
        for cn in range(1, NSQ):
            queue_proj_chunk(0, cn)
        for cn in range(NSQ):
            queue_proj_chunk(1, cn)

        # j-steps per att chunk: 4,8,12,16 (x2 batches) = 80 total.
        # bg items: 7 per proj chunk x 7 remaining = 49, + 8 outproj
        # pieces per flushed chunk (queued by emit_flush as we go).
        for i in range(NSQ):
            att_chunk(0, i, bg_per_j=2)
        b1_pace = {3: 1, 2: 1, 1: 2, 0: 2}
        for i in range(NSQ - 1, -1, -1):
            att_chunk(1, i, bg_per_j=b1_pace[i])
        drain_bg(len(bg))


def get_module(repeat=1):
    key = ("nc", repeat)
    if key not in _module_cache:
        m = _build_module(repeat=repeat)
        m.compile()
        _module_cache[key] = m
    return _module_cache[key]


def make_in_maps(x, qw, qb, kw, kb, vw, vb, ow):
    import ml_dtypes
    mmdt = np.dtype(np.float16)
    fp8dt = mybir.dt.np(FP8)
    xT32 = np.ascontiguousarray(x.transpose(0, 2, 1))  # [B, D, S]
    xT = xT32.astype(mmdt)
    tp = np.arange(P, dtype=np.int64)[:, None]
    f1 = np.arange(P, dtype=np.int64)[None, :]
    f2 = np.arange(2 * P, dtype=np.int64)[None, :]
    t128 = (tp <= f1).astype(mmdt)
    t256 = (tp <= f2 - P).astype(mmdt)
    maskm = np.concatenate([t128, t256], axis=1)
    in_maps = []
    for c in range(NCORES):
        sl = slice(c * FPC, (c + 1) * FPC)
        wT = np.concatenate(
            [qw[sl, :].T, kw[sl, :].T, vw[sl, :].T], axis=1)
        qb_eff = qb[sl] * (W8_SCALE if FP8_QK else 1.0)
        m = {
            "xT": xT,
            "wT": np.ascontiguousarray(wT).astype(mmdt),
            "qb": np.ascontiguousarray(qb_eff.reshape(FPC, 1)).astype(np.float32),
            "owT": np.ascontiguousarray(ow[:, sl].T).astype(mmdt),
            "maskm": maskm,
        }
        if FP8_QK:
            m["x8"] = xT32.astype(fp8dt)
            w8 = np.concatenate(
                [qw[sl, :].T, kw[sl, :].T], axis=1) * W8_SCALE
            m["w8"] = np.ascontiguousarray(w8).astype(fp8dt)
        in_maps.append(m)
    return in_maps


def kernel(x, qw, qb, kw, kb, vw, vb, ow, ob, _trace=False):
    x = np.asarray(x, dtype=np.float32)
    qw = np.asarray(qw, dtype=np.float32)
    qb = np.asarray(qb, dtype=np.float32)
    kw = np.asarray(kw, dtype=np.float32)
    kb = np.asarray(kb, dtype=np.float32)
    vw = np.asarray(vw, dtype=np.float32)
    vb = np.asarray(vb, dtype=np.float32)
    ow = np.asarray(ow, dtype=np.float32)
    ob = np.asarray(ob, dtype=np.float32)

    nc = get_module()
    in_maps = make_in_maps(x, qw, qb, kw, kb, vw, vb, ow)
    res = run_bass_kernel_spmd(
        nc, in_maps, core_ids=list(range(NCORES)), trace=_trace
    )
    acc = np.zeros((B, S, D), dtype=np.float64)
    for r in res.results:
        acc += r["out"].astype(np.float64)
    # host-side bias folding: ob' = ob + ow @ vb (see module docstring)
    ob_eff = ob.astype(np.float64) + ow.astype(np.float64) @ vb.astype(np.float64)
    out = (acc + ob_eff).astype(np.float32)
    if _trace:
        kernel.last_results = res
    return out
